# revision 35
# baseline (speedup 1.0000x reference)
"""MoE (noisy top-2 gate, 8 experts) Trainium2 kernel.

Strategy (expert-parallel, per the sharding hint):
  - The tiny gate (two [N,512]@[512,8] matmuls + softplus + top-2 + softmax)
    is evaluated on the host in float64 as part of the dispatch/routing step;
    the routing decision determines the all-to-all (here: a host-side
    gather by expert id, since kernel() receives the full input).
  - Each of the 8 NeuronCores holds ONE expert's weights and runs the FFN
    (relu(x@W1+b1)@W2, scaled by the gate weight) over the tokens routed to
    its expert, padded to a common capacity C=2048 (SPMD: one program,
    per-core data).  Matmuls run on the PE array in bf16 with fp32 PSUM
    accumulation.  Tokens routed past an expert's 2048 slots (~1% of slots)
    are computed exactly on the host.
  - The host then combines: out[token] = sum over its 2 slots + g*b2 terms
    (softmax weights of the chosen experts sum to 1, b2 handled exactly).

Schedule notes (the bf16 body runs at the ~223ns/MM issue roofline, so the
levers are the mixed-precision last chunk, the ramp-in and the drain):
  - Mixed precision by gate weight: the host orders each expert's slots by
    g DESCENDING, so the last 512-slot chunk holds the smallest-g slots.
    That chunk's mm1 runs as fp8(e4m3) DoubleRow matmuls (contraction 256
    per instruction = 2x bf16 throughput, ~7us off the PE stream); its
    error contribution is bounded by g* x fp8-noise with g* ~0.42.
    Measured end-to-end relmax 1.74e-2 vs the 2e-2 gate (sim: 1.77e-2;
    bf16-everywhere is 3.9e-3).  x8/W1x64 fp8 scaling is undone by the
    activation's scale=1/512.  The fp8 operands are quantized on the HOST
    (device bf16->fp8 re-rounding of W1 costs another 1.4e-3 of margin —
    rejected), and their ~1.1MB ships as extra staged input; the unused
    bf16 x columns of the last chunk are NOT staged (528KB back), since
    extra staging traffic contends with the kernel's own ramp DMAs
    (measured: 2-4us of PE stalls when +1.1MB is staged AND the fp8
    tensors are fetched during the ramp; their dma_starts are emitted at
    chunk 2 so the transfers ride the ~30us idle window instead).
  - Two HWDGE rings (sync + scalar): scalar carries W1 per-d rows then the
    two W2 halves then x-chunks; sync carries the four x-chunk-0 tiles,
    the packed b1+gate tensor, and y stores.  (Finer-grained parallel
    piece schedules were measured SLOWER: per-queue rate drops with line
    size and the extra per-matmul semaphore waits bubble the PE pipe.)
  - A PE warmup (dummy matmuls, no DMA deps) covers the wait until the
    first (W1 d-row, x tile) pair lands, then chunk-0's mm1 runs d-major
    with 8 open PSUM groups so compute paces the remaining W1 arrivals.
    The warmup deliberately ends ~1us short of data-ready: the idle gap
    re-arms the HAM clock throttle, so the d-major pass runs at the cold
    1.2GHz rate — which matches the DMA delivery rate and makes the run
    insensitive to multi-us DMA jitter.
  - W2 is pre-arranged on the host to the [128, f, d] SBUF layout so its
    DMA is a contiguous 2x1MB stream instead of a strided rearrange.
  - y returns in bf16.  The very last tile is computed as FOUR 128-column
    accumulation chains (warm 128-col matmuls run at full rate, so PE
    time is unchanged) with DISJOINT yt tiles: same-tile readers
    serialize ~90ns after the prior reader completes (measured), so
    per-chain tiles let each g-mul+store launch as soon as its chain
    stops, leaving only the last ~0.9us chain's drain exposed before the
    end-of-kernel barrier.

The fixed gate noise (jax.random.normal(PRNGKey(42), [4,2048,8])) is
reproduced bit-compatibly with a pure-numpy threefry2x32 + XLA's f32 erfinv
polynomial (verified: max |diff| < 5e-7 vs jax, while the smallest top-2
routing margin on this problem's data is ~3e-5).
"""

import os
from contextlib import ExitStack

import numpy as np
import ml_dtypes

import concourse.bacc as bacc
import concourse.bass as bass
import concourse.mybir as mybir
import concourse.tile as tile
from concourse.bass_utils import run_bass_kernel_spmd
from concourse.vector_clock import ScopedClock


def _lean_drain_and_barrier(self, tick_clock, wait_clock):
    """TileContext teardown minus the on-device semaphore range-clear.

    The stock teardown is: output-completion waits -> all-engine barrier ->
    gpsimd dma_reset+sem_clear of the whole sem range (a ~4us round trip
    through the software DMA queue) -> second all-engine barrier.  The
    range-clear only matters when another tile context (or a later BIR
    kernel) reuses the semaphores afterwards; this program has exactly one
    tile context at its very end, and the program preamble re-initializes
    every semaphore on each NEFF execution.  Dropping it removes ~5us of
    pure epilogue from every run.  Host-side bookkeeping (freeing the sem
    ids) is kept so compile-time allocation stays consistent.
    """
    drain_inst = self.nc.sync.drain()
    wait_clock.add_sem_waits(
        drain_inst.ins, ScopedClock({None: tick_clock.global_clock})
    )
    # No all-engine barrier here either: output completion is carried by
    # the sync drain's waits above, and the program epilogue (emitted by
    # Bacc after the kernel call) ends with its own all-engine rendezvous.
    # Each barrier round costs ~1-2us of serialized cross-engine semaphore
    # propagation, so one round instead of three.
    popped = self.nc._tile_sem_poison_stack.pop()
    assert popped is self._sem_poison
    sems = list(self.sems.allocated().values())
    sem_nums = [s if isinstance(s, int) else s.num for s in sems]
    self.nc._state.prepend_free_semaphores(sem_nums)


tile.TileContext._drain_and_barrier = _lean_drain_and_barrier

_B, _T, _D, _E, _K = 4, 2048, 512, 8, 2
_FF = 4 * _D
_N = _B * _T
_C = 2048  # device capacity (slots per expert); overflow computed on host

# matmul dtype mode: "bf16" | "f32" | "f32r"
_MODE = os.environ.get("MOE_MM_DTYPE", "bf16")
_TRACE = bool(int(os.environ.get("MOE_TRACE", "0")))
# PE warmup matmuls (107ns each, cold).  Deliberately ends ~1us BEFORE the
# expected data-ready (~11.6us): the small idle gap resets the HAM activity
# window, so chunk-0's d-major pass runs at the cold 1.2GHz clock — which
# matches the w1 DMA delivery rate.  A gapless (longer) warmup flips the
# clock early and the warm pass then stalls on w1 arrivals, exposing the
# run to multi-us DMA jitter (measured: higher variance, same-or-worse
# mean).  Cold pacing absorbs that jitter for free.
_WARM = int(os.environ.get("MOE_WARM", "34"))
# fp8 (e4m3, DoubleRow) mm1 for the last chunk — the C/4 smallest-gate
# slots per expert.  Simulated end-to-end relmax 1.77e-2 vs the 2e-2 gate
# (bf16 everywhere: 3.9e-3); saves ~7us of PE stream time.
_FP8 = bool(int(os.environ.get("MOE_FP8", "1")))

LAST_RESULTS = None  # BassKernelResults of the most recent device run


# ---------------------------------------------------------------------------
# Fixed gate noise: jax.random.normal(jax.random.PRNGKey(42), (4, 2048, 8))
# ---------------------------------------------------------------------------

def _threefry2x32(k0, k1, x0, x1):
    R0 = [13, 15, 26, 6]
    R1 = [17, 29, 16, 24]
    ks0, ks1 = np.uint32(k0), np.uint32(k1)
    ks2 = np.uint32(ks0 ^ ks1 ^ np.uint32(0x1BD11BDA))
    x0 = (x0 + ks0).astype(np.uint32)
    x1 = (x1 + ks1).astype(np.uint32)

    def rotl(v, d):
        return ((v << np.uint32(d)) | (v >> np.uint32(32 - d))).astype(np.uint32)

    ks = [ks0, ks1, ks2]
    for i in range(5):
        for r in R0 if i % 2 == 0 else R1:
            x0 = (x0 + x1).astype(np.uint32)
            x1 = rotl(x1, r)
            x1 = (x1 ^ x0).astype(np.uint32)
        x0 = (x0 + ks[(i + 1) % 3]).astype(np.uint32)
        x1 = (x1 + ks[(i + 2) % 3] + np.uint32(i + 1)).astype(np.uint32)
    return x0, x1


def _erfinv_f32(x):
    # XLA's single-precision ErfInv polynomial (Giles), evaluated in fp32.
    x = x.astype(np.float32)
    w = (-np.log1p((-x * x).astype(np.float32))).astype(np.float32)
    w1 = (w - np.float32(2.5)).astype(np.float32)
    p = np.full_like(x, np.float32(2.81022636e-08))
    for c in (3.43273939e-07, -3.5233877e-06, -4.39150654e-06, 0.00021858087,
              -0.00125372503, -0.00417768164, 0.246640727, 1.50140941):
        p = (p * w1 + np.float32(c)).astype(np.float32)
    w2 = (np.sqrt(w).astype(np.float32) - np.float32(3.0)).astype(np.float32)
    q = np.full_like(x, np.float32(-0.000200214257))
    for c in (0.000100950558, 0.00134934322, -0.00367342844, 0.00573950773,
              -0.0076224613, 0.00943887047, 1.00167406, 2.83297682):
        q = (q * w2 + np.float32(c)).astype(np.float32)
    return np.where(w < np.float32(5.0), p * x, q * x).astype(np.float32)


_NOISE_CACHE = None


def _gate_noise():
    """float32 [N, E] == jax.random.normal(PRNGKey(42), (B,T,E)).reshape(N,E)."""
    global _NOISE_CACHE
    if _NOISE_CACHE is None:
        n = _N * _E
        o0, o1 = _threefry2x32(0, 42, np.zeros(n, np.uint32),
                               np.arange(n, dtype=np.uint32))
        bits = o0 ^ o1
        fl = ((bits >> np.uint32(9)) | np.uint32(0x3F800000)).view(np.float32) \
            - np.float32(1.0)
        lo = np.nextafter(np.float32(-1), np.float32(0))
        hi = np.float32(1.0)
        u = np.maximum(lo, (fl * (hi - lo) + lo).astype(np.float32))
        _NOISE_CACHE = (np.float32(np.sqrt(2.0)) * _erfinv_f32(u)).reshape(_N, _E)
    return _NOISE_CACHE


# ---------------------------------------------------------------------------
# Device program: per-core expert FFN over C token slots
# ---------------------------------------------------------------------------

_KERNEL_CACHE = {}


def _build_device_kernel(C, mode, fp8=False):
    """One-expert FFN: y[c, :] = g[c] * relu(x[c] @ W1 + b1) @ W2  for C slots."""
    f32 = mybir.dt.float32
    dt_in = mybir.dt.bfloat16 if mode == "bf16" else f32
    dt_y = mybir.dt.bfloat16 if mode == "bf16" else f32
    nD, nF = _D // 128, _FF // 128  # 4, 16
    nTT = C // 128
    CH = 512
    assert C % CH == 0
    n_chunks = C // CH

    nc = bacc.Bacc("TRN2", target_bir_lowering=False, debug=False, num_devices=_E)
    # with fp8 the last chunk's x arrives only as xt8 — don't stage the
    # unused bf16 columns (528KB less input staging per core)
    xt_cols = C - CH if fp8 else C
    xt_d = nc.dram_tensor("xt", [_D, xt_cols], dt_in, kind="ExternalInput").ap()
    w1_d = nc.dram_tensor("w1", [_D, _FF], dt_in, kind="ExternalInput").ap()
    # w2 arrives host-pre-arranged to the SBUF layout [128, f*d]
    w2_d = nc.dram_tensor("w2t", [128, nF * _D], dt_in,
                          kind="ExternalInput").ap()
    # b1 (transposed per-f-tile) and per-slot gate weights, packed so they
    # ride one DMA (each dma_start costs ~0.7us of ring-sequencer issue)
    bg_d = nc.dram_tensor("bg", [128, nF + nTT], f32, kind="ExternalInput").ap()
    y_d = nc.dram_tensor("y", [C, _D], dt_y, kind="ExternalOutput").ap()
    if fp8:
        # fp8(e4m3) copies for the LAST chunk's mm1 (the C/4 smallest-gate
        # slots per expert — host sorts slots by gate weight): x scaled x8,
        # W1 scaled x64, exactly as quantized on the host; the DoubleRow
        # matmul contracts 256 rows per instruction = 2x bf16 throughput.
        fp8dt = mybir.dt.float8e4
        xt8_d = nc.dram_tensor("xt8", [_D, CH], fp8dt,
                               kind="ExternalInput").ap()
        w18_d = nc.dram_tensor("w18", [_D, _FF], fp8dt,
                               kind="ExternalInput").ap()

    relu = mybir.ActivationFunctionType.Relu

    def mm(ap):
        return ap.bitcast(mybir.dt.float32r) if mode == "f32r" else ap

    # chunk-0 d-major pass covers f0..7 with 8 open PSUM groups (all banks;
    # the warmup tile's bank is recycled for the last group — the pool
    # serializes that on warmup completion, which is long past by then)
    NDM = 8

    with tile.TileContext(nc) as tc, ExitStack() as ctx:
        const = ctx.enter_context(tc.tile_pool(name="const", bufs=1))
        xpool = ctx.enter_context(tc.tile_pool(name="xc", bufs=2))
        hpool = ctx.enter_context(tc.tile_pool(name="ht", bufs=2))
        # One PSUM pool = all 8 banks.  Chunk-0's d-major pass keeps NDM=8
        # accumulation groups open at once; everywhere else only 1-2 tiles
        # are live, so mm1/mm2 share the pool.
        ps_pool = ctx.enter_context(tc.tile_pool(name="ps", bufs=8, space="PSUM"))
        ypool = ctx.enter_context(tc.tile_pool(name="yo", bufs=4))

        # PE warmup: dummy matmuls with no DMA dependency keep the PE busy
        # while the input DMAs land.  The memset runs on the (otherwise
        # idle) gpsimd engine so neither DMA ring's sequencer is delayed.
        # See _WARM above for why it deliberately ends just short of the
        # data-ready time.
        wsrc = const.tile([128, 128], dt_in, tag="warm_src")
        nc.gpsimd.memset(wsrc[:], 0.0)
        wps = ps_pool.tile([128, CH], f32, tag="ps")
        for _ in range(_WARM):
            nc.tensor.matmul(wps[:, :128], lhsT=mm(wsrc[:]), rhs=mm(wsrc[:]),
                             start=True, stop=True)

        # --- Head DMAs -----------------------------------------------------
        # DMA cost model (measured over many variants): each dma_start
        # occupies its ring's sequencer for ~0.7us (DIRECT2D descriptor
        # generation), and a [128, X] load then takes ~3-4.5us almost
        # regardless of X or striding (per-DMA throughput ~125GB/s, no
        # intra-DMA engine parallelism).  Splitting the head loads only
        # adds issue overhead — the baseline-style big loads are at the
        # latency floor; the win is a warmup sized to hand off to the real
        # stream with no PE-idle gap (a gap resets the HAM activity window
        # and costs ~2us of half-clock matmuls).
        # ring S (scalar): w1[d0..d3], then w2 (both halves — their kicks
        #                  chain behind w1 completions, so w1 streams get
        #                  the HBM bandwidth first), x-chunks 1, 3
        # ring Y (sync):   xt0[d0..d3], b1+g, x-chunk 2, all y stores
        w1sb = [const.tile([128, _FF], dt_in, tag=f"w1_{d}", name=f"w1_{d}")
                for d in range(nD)]
        for d in range(nD):
            nc.scalar.dma_start(w1sb[d][:],
                                w1_d[d * 128:(d + 1) * 128, :])

        xts0 = []
        for d in range(nD):
            t = xpool.tile([128, CH], dt_in, tag=f"xt0_{d}", name=f"xt0_{d}",
                           bufs=1)
            nc.sync.dma_start(t[:], xt_d[d * 128:(d + 1) * 128, 0:CH])
            xts0.append(t)
        bgsb = const.tile([128, nF + nTT], f32, tag="bg")
        nc.sync.dma_start(bgsb[:], bg_d[:])
        b1sb = bgsb[:, :nF]
        gsb = bgsb[:, nF:]

        def w1sl(d, f):
            return w1sb[d][:, f * 128:(f + 1) * 128]

        w2sb = const.tile([128, nF, _D], dt_in, tag="w2")
        w2v = w2_d.rearrange("p (f j) -> p f j", j=_D)
        # both halves on scalar, strictly BEHIND the w1 rows: any second
        # stream drawing HBM during the w1 window slows the d-major pass
        # (measured 3-6us stalls with w2 on sync in parallel)
        nc.scalar.dma_start(w2sb[:, :nF // 2, :], w2v[:, :nF // 2, :])
        nc.scalar.dma_start(w2sb[:, nF // 2:, :], w2v[:, nF // 2:, :])

        def load_x_chunk(s, eng):
            base = s * CH
            t = xpool.tile([128, nD, CH], dt_in, tag="xt", name=f"xt_c{s}")
            eng.dma_start(
                t[:],
                xt_d[:, base:base + CH].rearrange("(d p) c -> p d c", p=128))
            return t

        xts_next = load_x_chunk(1, nc.scalar) if n_chunks > 1 else None

        if fp8:
            # fp8 tensors for the last chunk's mm1.  Their dma_starts are
            # emitted at the start of chunk 2 so they queue BEHIND every
            # ramp-critical transfer on the rings (w1/w2/x-chunks) and
            # transfer in the idle window at ~30-36us; first use is at ~3/4
            # of the run.  (Issued at head time they contend with the w1
            # stream and stall the d-major ramp — measured 2-4us of PE
            # gaps; the gpsimd SWDGE queue triggers eagerly, so parking
            # them there does not defer the transfers either.)
            fp8dt = mybir.dt.float8e4
            w18sb = const.tile([128, nD, _FF], fp8dt, tag="w18")
            w18v = w18_d.rearrange("(d p) f -> p d f", p=128)
            xt8sb = xpool.tile([128, nD, CH], fp8dt, tag="xt8", bufs=1)

            def emit_fp8_loads():
                nc.scalar.dma_start(w18sb[:, :nD // 2, :],
                                    w18v[:, :nD // 2, :])
                nc.sync.dma_start(w18sb[:, nD // 2:, :],
                                  w18v[:, nD // 2:, :])
                nc.scalar.dma_start(
                    xt8sb[:],
                    xt8_d.rearrange("(d p) c -> p d c", p=128))

        for s in range(n_chunks):
            base = s * CH
            fp8_chunk = fp8 and s == n_chunks - 1
            if s == 0:
                xsl = lambda d: xts0[d][:]
            elif not fp8_chunk:
                xts = xts_next
                if s + 1 < n_chunks and not (fp8 and s + 1 == n_chunks - 1):
                    xts_next = load_x_chunk(s + 1, nc.sync if s == 1 else nc.scalar)
                xsl = lambda d: xts[:, d, :]
            if fp8 and s == 2:
                emit_fp8_loads()
            ht3 = hpool.tile([128, nF, CH], dt_in, tag="ht3", name=f"ht3_{s}")
            ndm = NDM if s == 0 else 0
            if ndm:
                # d-major over the first `ndm` f's with `ndm` open PSUM
                # groups: step d needs only (xt0[d], w1[d]), so compute
                # starts as soon as the first pair lands and keeps pace
                # with the arrival of the rest.
                phs = [ps_pool.tile([128, CH], f32, tag="ps", name=f"ph_dm{j}")
                       for j in range(ndm)]
                for di, d in enumerate(range(nD)):
                    for j in range(ndm):
                        nc.tensor.matmul(
                            phs[j][:],
                            lhsT=mm(w1sl(d, j)),
                            rhs=mm(xsl(d)),
                            start=(di == 0),
                            stop=(di == nD - 1),
                        )
                for j in range(ndm):
                    nc.scalar.activation(ht3[:, j, :], phs[j][:], relu,
                                         bias=b1sb[:, j:j + 1])
            for f in range(ndm, nF):
                ph = ps_pool.tile([128, CH], f32, tag="ps", name=f"ph_{s}_{f}")
                if fp8_chunk:
                    # DoubleRow fp8: each instruction contracts a PAIR of
                    # 128-row d-tiles (lhsT/rhs dim1 = k-tiles) at 2x the
                    # bf16 rate; PSUM holds 512*(x@W1) (x scaled x8, W1
                    # x64), un-scaled in the activation.
                    for j in range(nD // 2):
                        nc.tensor.matmul(
                            ph[:],
                            lhsT=w18sb[:, 2 * j:2 * j + 2,
                                       f * 128:(f + 1) * 128],
                            rhs=xt8sb[:, 2 * j:2 * j + 2, :],
                            perf_mode=mybir.MatmulPerfMode.DoubleRow,
                            start=(j == 0),
                            stop=(j == nD // 2 - 1),
                        )
                    nc.scalar.activation(ht3[:, f, :], ph[:], relu,
                                         bias=b1sb[:, f:f + 1],
                                         scale=1.0 / 512.0)
                else:
                    for d in range(nD):
                        nc.tensor.matmul(
                            ph[:],
                            lhsT=mm(w1sl(d, f)),
                            rhs=mm(xsl(d)),
                            start=(d == 0),
                            stop=(d == nD - 1),
                        )
                    nc.scalar.activation(ht3[:, f, :], ph[:], relu,
                                         bias=b1sb[:, f:f + 1])
            # mm2: y[tt] = g[tt] * (hT[:, tt]^T @ W2)  -> [128 tok, 512 d]
            for t_ in range(CH // 128):
                tt = base // 128 + t_
                last_tile = (s == n_chunks - 1) and (t_ == CH // 128 - 1)
                # The very last tile is computed as FOUR 128-column
                # accumulation chains (a warm 128-col matmul runs at full
                # rate, so total PE time is unchanged) with DISJOINT yt
                # tiles: same-tile readers serialize ~90ns after the prior
                # reader completes (measured), so per-chain tiles let each
                # mul+store launch as soon as its chain stops, overlapping
                # the remaining chains' matmuls.  Only the last ~0.9us
                # chain's drain is exposed before the end-of-kernel barrier.
                col_splits = tuple(
                    (k * 128, (k + 1) * 128) for k in range(_D // 128)) \
                    if last_tile else ((0, _D),)
                for ci, (c0, c1) in enumerate(col_splits):
                    py = ps_pool.tile([128, c1 - c0], f32, tag="ps",
                                      name=f"py_{tt}_{ci}")
                    for f in range(nF):
                        nc.tensor.matmul(
                            py[:],
                            lhsT=mm(ht3[:, f, t_ * 128:(t_ + 1) * 128]),
                            rhs=mm(w2sb[:, f, c0:c1]),
                            start=(f == 0),
                            stop=(f == nF - 1),
                        )
                    if last_tile:
                        yt = ypool.tile([128, c1 - c0], dt_y, tag=f"ytf{ci}",
                                        name=f"yt_{tt}_{ci}", bufs=1)
                        nc.vector.tensor_scalar_mul(yt[:], py[:],
                                                    gsb[:, tt:tt + 1])
                        eng = nc.sync if ci % 2 == 0 else nc.scalar
                        eng.dma_start(y_d[tt * 128:(tt + 1) * 128, c0:c1],
                                      yt[:])
                    else:
                        yt = ypool.tile([128, c1 - c0], dt_y, tag="yt",
                                        name=f"yt_{tt}_{ci}")
                        nc.vector.tensor_scalar_mul(yt[:], py[:],
                                                    gsb[:, tt:tt + 1])
                        # alternate rings so consecutive tile stores never
                        # queue behind each other
                        eng = nc.sync if tt % 2 == 0 else nc.scalar
                        eng.dma_start(y_d[tt * 128:(tt + 1) * 128, c0:c1],
                                      yt[:])

    nc.compile()
    return nc


def _get_device_kernel(C, mode, fp8=False):
    key = (C, mode, fp8)
    if key not in _KERNEL_CACHE:
        _KERNEL_CACHE[key] = _build_device_kernel(C, mode, fp8)
    return _KERNEL_CACHE[key]


# ---------------------------------------------------------------------------
# Host: gate, routing, dispatch, combine
# ---------------------------------------------------------------------------

def _route(x2, Wg_w, Wg_b, Wn_w, Wn_b):
    """float64 gate -> per-token top-2 experts and softmax weights."""
    x64 = x2.astype(np.float64)
    noise = _gate_noise().astype(np.float64)
    softplus = np.logaddexp(0.0, x64 @ Wn_w.astype(np.float64)
                            + Wn_b.astype(np.float64))
    Hx = (x64 @ Wg_w.astype(np.float64) + Wg_b.astype(np.float64)) \
        + noise * softplus
    order = np.argsort(-Hx, axis=1)
    e1, e2 = order[:, 0], order[:, 1]
    rows = np.arange(_N)
    v1, v2 = Hx[rows, e1], Hx[rows, e2]
    g1 = 1.0 / (1.0 + np.exp(v2 - v1))
    g2 = 1.0 - g1
    return e1, e2, g1.astype(np.float32), g2.astype(np.float32)


def kernel(x, Wg_w, Wg_b, Wn_w, Wn_b, W1, b1, W2, b2):
    global LAST_RESULTS
    x = np.asarray(x, dtype=np.float32)
    Wg_w = np.asarray(Wg_w, dtype=np.float32)
    Wg_b = np.asarray(Wg_b, dtype=np.float32)
    Wn_w = np.asarray(Wn_w, dtype=np.float32)
    Wn_b = np.asarray(Wn_b, dtype=np.float32)
    W1 = np.asarray(W1, dtype=np.float32)
    b1 = np.asarray(b1, dtype=np.float32)
    W2 = np.asarray(W2, dtype=np.float32)
    b2 = np.asarray(b2, dtype=np.float32)
    assert x.shape == (_B, _T, _D), x.shape

    x2 = np.ascontiguousarray(x.reshape(_N, _D))
    e1, e2, g1, g2 = _route(x2, Wg_w, Wg_b, Wn_w, Wn_b)

    # Entries: one (token, expert, gateweight) pair per routed slot.
    # Within each expert, slots are ordered by gate weight DESCENDING so
    # the last chunk holds the smallest-g slots — their mm1 runs in fp8
    # (error contribution is bounded by g * fp8-error), and any capacity
    # overflow (the very smallest g) falls back to the exact host path.
    ent_e = np.concatenate([e1, e2])
    ent_tok = np.concatenate([np.arange(_N), np.arange(_N)])
    ent_g = np.concatenate([g1, g2])
    perm = np.lexsort((-ent_g, ent_e))
    counts = np.bincount(ent_e, minlength=_E)
    starts = np.concatenate([[0], np.cumsum(counts)[:-1]])

    # Device capacity: fixed C slots per expert; entries past the cap
    # (expected ~1% of slots when counts exceed the mean) fall back to an
    # exact host-side FFN.
    C = _C
    nTT = C // 128

    # Global slot id for each entry (expert * C + position within expert);
    # overflow entries get the sentinel slot _E*C (a zero row on combine).
    pos_sorted = np.arange(2 * _N) - starts[ent_e[perm]]
    over = pos_sorted >= C
    slot_sorted = np.where(over, _E * C, ent_e[perm] * C + pos_sorted)
    slots = np.empty(2 * _N, dtype=np.int64)
    slots[perm] = slot_sorted
    tok_sorted = ent_tok[perm]

    # Per-slot gate weights, flattened over all cores.
    gflat = np.zeros(_E * C + 1, dtype=np.float32)
    gflat[slot_sorted] = ent_g[perm]
    gflat = gflat[:_E * C]

    cast = (lambda a: np.ascontiguousarray(a, dtype=ml_dtypes.bfloat16)) \
        if _MODE == "bf16" else (lambda a: np.ascontiguousarray(a, dtype=np.float32))

    fp8 = _FP8 and _MODE == "bf16"
    CH = 512
    in_maps = []
    for e in range(_E):
        cnt = min(int(counts[e]), C)
        toks = tok_sorted[starts[e]:starts[e] + cnt]
        xg = np.zeros((C, _D), dtype=np.float32)
        xg[:cnt] = x2[toks]
        im = {
            "xt": cast(xg[:C - CH].T if fp8 else xg.T),
            "w1": cast(W1[e]),
            # pre-arranged to the device SBUF layout [128, f, d]
            "w2t": cast(W2[e].reshape(_FF // 128, 128, _D)
                        .transpose(1, 0, 2).reshape(128, -1)),
            "bg": np.ascontiguousarray(np.concatenate([
                b1[e].reshape(_FF // 128, 128).T,
                gflat[e * C:(e + 1) * C].reshape(nTT, 128).T,
            ], axis=1), dtype=np.float32),
        }
        if fp8:
            im["xt8"] = np.ascontiguousarray(
                (xg[C - CH:].T * np.float32(8.0))
                .astype(ml_dtypes.float8_e4m3))
            im["w18"] = np.ascontiguousarray(
                (W1[e] * np.float32(64.0)).astype(ml_dtypes.float8_e4m3))
        in_maps.append(im)

    nc = _get_device_kernel(C, _MODE, fp8)
    res = run_bass_kernel_spmd(nc, in_maps, list(range(_E)), trace=_TRACE)
    LAST_RESULTS = res

    y_all = np.concatenate(
        [np.asarray(res.results[e]["y"], dtype=np.float32) for e in range(_E)]
        + [np.zeros((1, _D), dtype=np.float32)], axis=0)  # [E*C + 1, D]
    out = y_all[slots[:_N]] + y_all[slots[_N:]]

    # Exact host FFN for capacity-overflow entries (past slot C of an expert).
    if over.any():
        ov_tok = tok_sorted[over]
        ov_e = ent_e[perm][over]
        ov_g = ent_g[perm][over]
        for e in np.unique(ov_e):
            m = ov_e == e
            t = ov_tok[m]
            h = np.maximum(x2[t] @ W1[e] + b1[e], 0.0)
            out[t] += ov_g[m][:, None] * (h @ W2[e])

    # b2 of the chosen experts (device computes g*(relu(.)@W2) without b2)
    if b2.any():
        out += g1[:, None] * b2[e1] + g2[:, None] * b2[e2]
    return out.reshape(_B, _T, _D).astype(np.float32)



# revision 40
# speedup vs baseline: 1.1783x; 1.1783x over previous
"""MoE (noisy top-2 gate, 8 experts) Trainium2 kernel.

Strategy (expert-parallel, per the sharding hint):
  - The tiny gate (two [N,512]@[512,8] matmuls + softplus + top-2 + softmax)
    is evaluated on the host in float64 as part of the dispatch/routing step;
    the routing decision determines the all-to-all (here: a host-side
    gather by expert id, since kernel() receives the full input).
  - Each of the 8 NeuronCores holds ONE expert's weights and runs the FFN
    (relu(x@W1+b1)@W2, scaled by the gate weight) over the tokens routed to
    its expert, padded to a common capacity C=2048 (SPMD: one program,
    per-core data).  Matmuls run on the PE array in bf16 with fp32 PSUM
    accumulation.  Tokens routed past an expert's 2048 slots (~1% of slots)
    are computed exactly on the host.
  - The host then combines: out[token] = sum over its 2 slots + g*b2 terms
    (softmax weights of the chosen experts sum to 1, b2 handled exactly).

Schedule notes (the bf16 body runs at the ~223ns/MM issue roofline, so the
levers are the mixed-precision last chunk, the ramp-in and the drain):
  - Mixed precision by gate weight: the host orders each expert's slots by
    g DESCENDING, so the last 512-slot chunk holds the smallest-g slots.
    That chunk's mm1 runs as fp8(e4m3) DoubleRow matmuls (contraction 256
    per instruction = 2x bf16 throughput, ~7us off the PE stream); its
    error contribution is bounded by g* x fp8-noise with g* ~0.42.
    Measured end-to-end relmax 1.74e-2 vs the 2e-2 gate (sim: 1.77e-2;
    bf16-everywhere is 3.9e-3).  x8/W1x64 fp8 scaling is undone by the
    activation's scale=1/512.  The fp8 operands are quantized on the HOST
    (device bf16->fp8 re-rounding of W1 costs another 1.4e-3 of margin —
    rejected), and their ~1.1MB ships as extra staged input; the unused
    bf16 x columns of the last chunk are NOT staged (528KB back), since
    extra staging traffic contends with the kernel's own ramp DMAs
    (measured: 2-4us of PE stalls when +1.1MB is staged AND the fp8
    tensors are fetched during the ramp; their dma_starts are emitted at
    chunk 2 so the transfers ride the ~30us idle window instead).
  - Two HWDGE rings (sync + scalar): scalar carries W1 per-d rows then the
    two W2 halves then x-chunks; sync carries the four x-chunk-0 tiles,
    the packed b1+gate tensor, and y stores.  (Finer-grained parallel
    piece schedules were measured SLOWER: per-queue rate drops with line
    size and the extra per-matmul semaphore waits bubble the PE pipe.)
  - A PE warmup (dummy matmuls, no DMA deps) covers the wait until the
    first (W1 d-row, x tile) pair lands, then chunk-0's mm1 runs d-major
    with 8 open PSUM groups so compute paces the remaining W1 arrivals.
    The warmup deliberately ends ~1us short of data-ready: the idle gap
    re-arms the HAM clock throttle, so the d-major pass runs at the cold
    1.2GHz rate — which matches the DMA delivery rate and makes the run
    insensitive to multi-us DMA jitter.
  - W2 is pre-arranged on the host to the [128, f, d] SBUF layout so its
    DMA is a contiguous 2x1MB stream instead of a strided rearrange.
  - y returns in bf16.  The very last tile is computed as FOUR 128-column
    accumulation chains (warm 128-col matmuls run at full rate, so PE
    time is unchanged) with DISJOINT yt tiles: same-tile readers
    serialize ~90ns after the prior reader completes (measured), so
    per-chain tiles let each g-mul+store launch as soon as its chain
    stops, leaving only the last ~0.9us chain's drain exposed before the
    end-of-kernel barrier.

The fixed gate noise (jax.random.normal(PRNGKey(42), [4,2048,8])) is
reproduced bit-compatibly with a pure-numpy threefry2x32 + XLA's f32 erfinv
polynomial (verified: max |diff| < 5e-7 vs jax, while the smallest top-2
routing margin on this problem's data is ~3e-5).
"""

import os
from contextlib import ExitStack

import numpy as np
import ml_dtypes

import concourse.bacc as bacc
import concourse.bass as bass
import concourse.mybir as mybir
import concourse.tile as tile
from concourse.bass_utils import run_bass_kernel_spmd
from concourse.vector_clock import ScopedClock


def _lean_drain_and_barrier(self, tick_clock, wait_clock):
    """TileContext teardown minus the on-device semaphore range-clear.

    The stock teardown is: output-completion waits -> all-engine barrier ->
    gpsimd dma_reset+sem_clear of the whole sem range (a ~4us round trip
    through the software DMA queue) -> second all-engine barrier.  The
    range-clear only matters when another tile context (or a later BIR
    kernel) reuses the semaphores afterwards; this program has exactly one
    tile context at its very end, and the program preamble re-initializes
    every semaphore on each NEFF execution.  Dropping it removes ~5us of
    pure epilogue from every run.  Host-side bookkeeping (freeing the sem
    ids) is kept so compile-time allocation stays consistent.
    """
    drain_inst = self.nc.sync.drain()
    wait_clock.add_sem_waits(
        drain_inst.ins, ScopedClock({None: tick_clock.global_clock})
    )
    # No all-engine barrier here either: output completion is carried by
    # the sync drain's waits above, and the program epilogue (emitted by
    # Bacc after the kernel call) ends with its own all-engine rendezvous.
    # Each barrier round costs ~1-2us of serialized cross-engine semaphore
    # propagation, so one round instead of three.
    popped = self.nc._tile_sem_poison_stack.pop()
    assert popped is self._sem_poison
    sems = list(self.sems.allocated().values())
    sem_nums = [s if isinstance(s, int) else s.num for s in sems]
    self.nc._state.prepend_free_semaphores(sem_nums)


tile.TileContext._drain_and_barrier = _lean_drain_and_barrier

_B, _T, _D, _E, _K = 4, 2048, 512, 8, 2
_FF = 4 * _D
_N = _B * _T
_C = 2048  # device capacity (slots per expert); overflow computed on host

# matmul dtype mode: "bf16" | "f32" | "f32r"
_MODE = os.environ.get("MOE_MM_DTYPE", "bf16")
_TRACE = bool(int(os.environ.get("MOE_TRACE", "0")))
# PE warmup matmuls (107ns each, cold).  Deliberately ends ~1us BEFORE the
# expected data-ready (~11.6us): the small idle gap resets the HAM activity
# window, so chunk-0's d-major pass runs at the cold 1.2GHz clock — which
# matches the w1 DMA delivery rate.  A gapless (longer) warmup flips the
# clock early and the warm pass then stalls on w1 arrivals, exposing the
# run to multi-us DMA jitter (measured: higher variance, same-or-worse
# mean).  Cold pacing absorbs that jitter for free.
_WARM = int(os.environ.get("MOE_WARM", "34"))
# fp8 (e4m3, DoubleRow) mm1 for the last chunk — the C/4 smallest-gate
# slots per expert.  Simulated end-to-end relmax 1.77e-2 vs the 2e-2 gate
# (bf16 everywhere: 3.9e-3); saves ~7us of PE stream time.
_FP8 = bool(int(os.environ.get("MOE_FP8", "1")))

LAST_RESULTS = None  # BassKernelResults of the most recent device run


# ---------------------------------------------------------------------------
# Fixed gate noise: jax.random.normal(jax.random.PRNGKey(42), (4, 2048, 8))
# ---------------------------------------------------------------------------

def _threefry2x32(k0, k1, x0, x1):
    R0 = [13, 15, 26, 6]
    R1 = [17, 29, 16, 24]
    ks0, ks1 = np.uint32(k0), np.uint32(k1)
    ks2 = np.uint32(ks0 ^ ks1 ^ np.uint32(0x1BD11BDA))
    x0 = (x0 + ks0).astype(np.uint32)
    x1 = (x1 + ks1).astype(np.uint32)

    def rotl(v, d):
        return ((v << np.uint32(d)) | (v >> np.uint32(32 - d))).astype(np.uint32)

    ks = [ks0, ks1, ks2]
    for i in range(5):
        for r in R0 if i % 2 == 0 else R1:
            x0 = (x0 + x1).astype(np.uint32)
            x1 = rotl(x1, r)
            x1 = (x1 ^ x0).astype(np.uint32)
        x0 = (x0 + ks[(i + 1) % 3]).astype(np.uint32)
        x1 = (x1 + ks[(i + 2) % 3] + np.uint32(i + 1)).astype(np.uint32)
    return x0, x1


def _erfinv_f32(x):
    # XLA's single-precision ErfInv polynomial (Giles), evaluated in fp32.
    x = x.astype(np.float32)
    w = (-np.log1p((-x * x).astype(np.float32))).astype(np.float32)
    w1 = (w - np.float32(2.5)).astype(np.float32)
    p = np.full_like(x, np.float32(2.81022636e-08))
    for c in (3.43273939e-07, -3.5233877e-06, -4.39150654e-06, 0.00021858087,
              -0.00125372503, -0.00417768164, 0.246640727, 1.50140941):
        p = (p * w1 + np.float32(c)).astype(np.float32)
    w2 = (np.sqrt(w).astype(np.float32) - np.float32(3.0)).astype(np.float32)
    q = np.full_like(x, np.float32(-0.000200214257))
    for c in (0.000100950558, 0.00134934322, -0.00367342844, 0.00573950773,
              -0.0076224613, 0.00943887047, 1.00167406, 2.83297682):
        q = (q * w2 + np.float32(c)).astype(np.float32)
    return np.where(w < np.float32(5.0), p * x, q * x).astype(np.float32)


_NOISE_CACHE = None


def _gate_noise():
    """float32 [N, E] == jax.random.normal(PRNGKey(42), (B,T,E)).reshape(N,E)."""
    global _NOISE_CACHE
    if _NOISE_CACHE is None:
        n = _N * _E
        o0, o1 = _threefry2x32(0, 42, np.zeros(n, np.uint32),
                               np.arange(n, dtype=np.uint32))
        bits = o0 ^ o1
        fl = ((bits >> np.uint32(9)) | np.uint32(0x3F800000)).view(np.float32) \
            - np.float32(1.0)
        lo = np.nextafter(np.float32(-1), np.float32(0))
        hi = np.float32(1.0)
        u = np.maximum(lo, (fl * (hi - lo) + lo).astype(np.float32))
        _NOISE_CACHE = (np.float32(np.sqrt(2.0)) * _erfinv_f32(u)).reshape(_N, _E)
    return _NOISE_CACHE


# ---------------------------------------------------------------------------
# Device program: per-core expert FFN over C token slots
# ---------------------------------------------------------------------------

_KERNEL_CACHE = {}


def _build_device_kernel(C, mode, fp8=False):
    """One-expert FFN: y[c, :] = g[c] * relu(x[c] @ W1 + b1) @ W2  for C slots."""
    f32 = mybir.dt.float32
    dt_in = mybir.dt.bfloat16 if mode == "bf16" else f32
    dt_y = mybir.dt.bfloat16 if mode == "bf16" else f32
    nD, nF = _D // 128, _FF // 128  # 4, 16
    nTT = C // 128
    CH = 512
    assert C % CH == 0
    n_chunks = C // CH

    nc = bacc.Bacc("TRN2", target_bir_lowering=False, debug=False, num_devices=_E)
    # with fp8 the last chunk's x arrives only as xt8 — don't stage the
    # unused bf16 columns (528KB less input staging per core)
    xt_cols = C - CH if fp8 else C
    xt_d = nc.dram_tensor("xt", [_D, xt_cols], dt_in, kind="ExternalInput").ap()
    w1_d = nc.dram_tensor("w1", [_D, _FF], dt_in, kind="ExternalInput").ap()
    # w2 arrives host-pre-arranged to the SBUF layout [128, f*d]
    w2_d = nc.dram_tensor("w2t", [128, nF * _D], dt_in,
                          kind="ExternalInput").ap()
    # b1 (transposed per-f-tile) and per-slot gate weights, packed so they
    # ride one DMA (each dma_start costs ~0.7us of ring-sequencer issue)
    bg_d = nc.dram_tensor("bg", [128, nF + nTT], f32, kind="ExternalInput").ap()
    y_d = nc.dram_tensor("y", [C, _D], dt_y, kind="ExternalOutput").ap()
    if fp8:
        # fp8(e4m3) copies for the LAST chunk's mm1 (the C/4 smallest-gate
        # slots per expert — host sorts slots by gate weight): x scaled x8,
        # W1 scaled x64, exactly as quantized on the host; the DoubleRow
        # matmul contracts 256 rows per instruction = 2x bf16 throughput.
        fp8dt = mybir.dt.float8e4
        xt8_d = nc.dram_tensor("xt8", [_D, CH], fp8dt,
                               kind="ExternalInput").ap()
        w18_d = nc.dram_tensor("w18", [_D, _FF], fp8dt,
                               kind="ExternalInput").ap()

    relu = mybir.ActivationFunctionType.Relu

    def mm(ap):
        return ap.bitcast(mybir.dt.float32r) if mode == "f32r" else ap

    # chunk-0 d-major pass covers f0..7 with 8 open PSUM groups (all banks;
    # the warmup tile's bank is recycled for the last group — the pool
    # serializes that on warmup completion, which is long past by then)
    NDM = 8

    with tile.TileContext(nc) as tc, ExitStack() as ctx:
        const = ctx.enter_context(tc.tile_pool(name="const", bufs=1))
        xpool = ctx.enter_context(tc.tile_pool(name="xc", bufs=2))
        hpool = ctx.enter_context(tc.tile_pool(name="ht", bufs=2))
        # One PSUM pool = all 8 banks.  Chunk-0's d-major pass keeps NDM=8
        # accumulation groups open at once; everywhere else only 1-2 tiles
        # are live, so mm1/mm2 share the pool.
        ps_pool = ctx.enter_context(tc.tile_pool(name="ps", bufs=8, space="PSUM"))
        ypool = ctx.enter_context(tc.tile_pool(name="yo", bufs=4))

        # PE warmup: dummy matmuls with no DMA dependency keep the PE busy
        # while the input DMAs land.  The memset runs on the (otherwise
        # idle) gpsimd engine so neither DMA ring's sequencer is delayed.
        # See _WARM above for why it deliberately ends just short of the
        # data-ready time.
        wsrc = const.tile([128, 128], dt_in, tag="warm_src")
        nc.gpsimd.memset(wsrc[:], 0.0)
        wps = ps_pool.tile([128, CH], f32, tag="ps")
        for _ in range(_WARM):
            nc.tensor.matmul(wps[:, :128], lhsT=mm(wsrc[:]), rhs=mm(wsrc[:]),
                             start=True, stop=True)

        # --- Head DMAs -----------------------------------------------------
        # DMA cost model (measured over many variants): each dma_start
        # occupies its ring's sequencer for ~0.7us (DIRECT2D descriptor
        # generation), and a [128, X] load then takes ~3-4.5us almost
        # regardless of X or striding (per-DMA throughput ~125GB/s, no
        # intra-DMA engine parallelism).  Splitting the head loads only
        # adds issue overhead — the baseline-style big loads are at the
        # latency floor; the win is a warmup sized to hand off to the real
        # stream with no PE-idle gap (a gap resets the HAM activity window
        # and costs ~2us of half-clock matmuls).
        # ring S (scalar): w1[d0..d3], then w2 (both halves — their kicks
        #                  chain behind w1 completions, so w1 streams get
        #                  the HBM bandwidth first), x-chunks 1, 3
        # ring Y (sync):   xt0[d0..d3], b1+g, x-chunk 2, all y stores
        w1sb = [const.tile([128, _FF], dt_in, tag=f"w1_{d}", name=f"w1_{d}")
                for d in range(nD)]
        for d in range(nD):
            nc.scalar.dma_start(w1sb[d][:],
                                w1_d[d * 128:(d + 1) * 128, :])

        xts0 = []
        for d in range(nD):
            t = xpool.tile([128, CH], dt_in, tag=f"xt0_{d}", name=f"xt0_{d}",
                           bufs=1)
            nc.sync.dma_start(t[:], xt_d[d * 128:(d + 1) * 128, 0:CH])
            xts0.append(t)
        bgsb = const.tile([128, nF + nTT], f32, tag="bg")
        nc.sync.dma_start(bgsb[:], bg_d[:])
        b1sb = bgsb[:, :nF]
        gsb = bgsb[:, nF:]

        def w1sl(d, f):
            return w1sb[d][:, f * 128:(f + 1) * 128]

        w2sb = const.tile([128, nF, _D], dt_in, tag="w2")
        w2v = w2_d.rearrange("p (f j) -> p f j", j=_D)
        # both halves on scalar, strictly BEHIND the w1 rows: any second
        # stream drawing HBM during the w1 window slows the d-major pass
        # (measured 3-6us stalls with w2 on sync in parallel)
        nc.scalar.dma_start(w2sb[:, :nF // 2, :], w2v[:, :nF // 2, :])
        nc.scalar.dma_start(w2sb[:, nF // 2:, :], w2v[:, nF // 2:, :])

        def load_x_chunk(s, eng):
            base = s * CH
            t = xpool.tile([128, nD, CH], dt_in, tag="xt", name=f"xt_c{s}")
            eng.dma_start(
                t[:],
                xt_d[:, base:base + CH].rearrange("(d p) c -> p d c", p=128))
            return t

        xts_next = load_x_chunk(1, nc.scalar) if n_chunks > 1 else None

        if fp8:
            # fp8 tensors for the last chunk's mm1.  Their dma_starts are
            # emitted at the start of chunk 2 so they queue BEHIND every
            # ramp-critical transfer on the rings (w1/w2/x-chunks) and
            # transfer in the idle window at ~30-36us; first use is at ~3/4
            # of the run.  (Issued at head time they contend with the w1
            # stream and stall the d-major ramp — measured 2-4us of PE
            # gaps; the gpsimd SWDGE queue triggers eagerly, so parking
            # them there does not defer the transfers either.)
            fp8dt = mybir.dt.float8e4
            w18sb = const.tile([128, nD, _FF], fp8dt, tag="w18")
            w18v = w18_d.rearrange("(d p) f -> p d f", p=128)
            xt8sb = xpool.tile([128, nD, CH], fp8dt, tag="xt8", bufs=1)

            def emit_fp8_loads():
                nc.scalar.dma_start(w18sb[:, :nD // 2, :],
                                    w18v[:, :nD // 2, :])
                nc.sync.dma_start(w18sb[:, nD // 2:, :],
                                  w18v[:, nD // 2:, :])
                nc.scalar.dma_start(
                    xt8sb[:],
                    xt8_d.rearrange("(d p) c -> p d c", p=128))

        for s in range(n_chunks):
            base = s * CH
            fp8_chunk = fp8 and s == n_chunks - 1
            if s == 0:
                xsl = lambda d: xts0[d][:]
            elif not fp8_chunk:
                xts = xts_next
                if s + 1 < n_chunks and not (fp8 and s + 1 == n_chunks - 1):
                    xts_next = load_x_chunk(s + 1, nc.sync if s == 1 else nc.scalar)
                xsl = lambda d: xts[:, d, :]
            if fp8 and s == 2:
                emit_fp8_loads()
            ht3 = hpool.tile([128, nF, CH], dt_in, tag="ht3", name=f"ht3_{s}")
            ndm = NDM if s == 0 else 0
            if ndm:
                # d-major over the first `ndm` f's with `ndm` open PSUM
                # groups: step d needs only (xt0[d], w1[d]), so compute
                # starts as soon as the first pair lands and keeps pace
                # with the arrival of the rest.
                phs = [ps_pool.tile([128, CH], f32, tag="ps", name=f"ph_dm{j}")
                       for j in range(ndm)]
                for di, d in enumerate(range(nD)):
                    for j in range(ndm):
                        nc.tensor.matmul(
                            phs[j][:],
                            lhsT=mm(w1sl(d, j)),
                            rhs=mm(xsl(d)),
                            start=(di == 0),
                            stop=(di == nD - 1),
                        )
                for j in range(ndm):
                    nc.scalar.activation(ht3[:, j, :], phs[j][:], relu,
                                         bias=b1sb[:, j:j + 1])
            for f in range(ndm, nF):
                ph = ps_pool.tile([128, CH], f32, tag="ps", name=f"ph_{s}_{f}")
                if fp8_chunk:
                    # DoubleRow fp8: each instruction contracts a PAIR of
                    # 128-row d-tiles (lhsT/rhs dim1 = k-tiles) at 2x the
                    # bf16 rate; PSUM holds 512*(x@W1) (x scaled x8, W1
                    # x64), un-scaled in the activation.
                    for j in range(nD // 2):
                        nc.tensor.matmul(
                            ph[:],
                            lhsT=w18sb[:, 2 * j:2 * j + 2,
                                       f * 128:(f + 1) * 128],
                            rhs=xt8sb[:, 2 * j:2 * j + 2, :],
                            perf_mode=mybir.MatmulPerfMode.DoubleRow,
                            start=(j == 0),
                            stop=(j == nD // 2 - 1),
                        )
                    nc.scalar.activation(ht3[:, f, :], ph[:], relu,
                                         bias=b1sb[:, f:f + 1],
                                         scale=1.0 / 512.0)
                else:
                    for d in range(nD):
                        nc.tensor.matmul(
                            ph[:],
                            lhsT=mm(w1sl(d, f)),
                            rhs=mm(xsl(d)),
                            start=(d == 0),
                            stop=(d == nD - 1),
                        )
                    nc.scalar.activation(ht3[:, f, :], ph[:], relu,
                                         bias=b1sb[:, f:f + 1])
            # mm2: y[tt] = g[tt] * (hT[:, tt]^T @ W2)  -> [128 tok, 512 d]
            for t_ in range(CH // 128):
                tt = base // 128 + t_
                last_tile = (s == n_chunks - 1) and (t_ == CH // 128 - 1)
                # The very last tile is computed as FOUR 128-column
                # accumulation chains (a warm 128-col matmul runs at full
                # rate, so total PE time is unchanged) with DISJOINT yt
                # tiles: same-tile readers serialize ~90ns after the prior
                # reader completes (measured), so per-chain tiles let each
                # mul+store launch as soon as its chain stops, overlapping
                # the remaining chains' matmuls.  Only the last ~0.9us
                # chain's drain is exposed before the end-of-kernel barrier.
                col_splits = tuple(
                    (k * 128, (k + 1) * 128) for k in range(_D // 128)) \
                    if last_tile else ((0, _D),)
                for ci, (c0, c1) in enumerate(col_splits):
                    py = ps_pool.tile([128, c1 - c0], f32, tag="ps",
                                      name=f"py_{tt}_{ci}")
                    for f in range(nF):
                        nc.tensor.matmul(
                            py[:],
                            lhsT=mm(ht3[:, f, t_ * 128:(t_ + 1) * 128]),
                            rhs=mm(w2sb[:, f, c0:c1]),
                            start=(f == 0),
                            stop=(f == nF - 1),
                        )
                    if last_tile:
                        yt = ypool.tile([128, c1 - c0], dt_y, tag=f"ytf{ci}",
                                        name=f"yt_{tt}_{ci}", bufs=1)
                        nc.vector.tensor_scalar_mul(yt[:], py[:],
                                                    gsb[:, tt:tt + 1])
                        eng = nc.sync if ci % 2 == 0 else nc.scalar
                        eng.dma_start(y_d[tt * 128:(tt + 1) * 128, c0:c1],
                                      yt[:])
                    else:
                        yt = ypool.tile([128, c1 - c0], dt_y, tag="yt",
                                        name=f"yt_{tt}_{ci}")
                        nc.vector.tensor_scalar_mul(yt[:], py[:],
                                                    gsb[:, tt:tt + 1])
                        # alternate rings so consecutive tile stores never
                        # queue behind each other
                        eng = nc.sync if tt % 2 == 0 else nc.scalar
                        eng.dma_start(y_d[tt * 128:(tt + 1) * 128, c0:c1],
                                      yt[:])

    nc.compile()
    return nc


def _get_device_kernel(C, mode, fp8=False):
    key = (C, mode, fp8)
    if key not in _KERNEL_CACHE:
        _KERNEL_CACHE[key] = _build_device_kernel(C, mode, fp8)
    return _KERNEL_CACHE[key]


# ---------------------------------------------------------------------------
# Host: gate, routing, dispatch, combine
# ---------------------------------------------------------------------------

def _route(x2, Wg_w, Wg_b, Wn_w, Wn_b):
    """float64 gate -> per-token top-2 experts and softmax weights."""
    x64 = x2.astype(np.float64)
    noise = _gate_noise().astype(np.float64)
    softplus = np.logaddexp(0.0, x64 @ Wn_w.astype(np.float64)
                            + Wn_b.astype(np.float64))
    Hx = (x64 @ Wg_w.astype(np.float64) + Wg_b.astype(np.float64)) \
        + noise * softplus
    order = np.argsort(-Hx, axis=1)
    e1, e2 = order[:, 0], order[:, 1]
    rows = np.arange(_N)
    v1, v2 = Hx[rows, e1], Hx[rows, e2]
    g1 = 1.0 / (1.0 + np.exp(v2 - v1))
    g2 = 1.0 - g1
    return e1, e2, g1.astype(np.float32), g2.astype(np.float32)


def kernel(x, Wg_w, Wg_b, Wn_w, Wn_b, W1, b1, W2, b2):
    global LAST_RESULTS
    x = np.asarray(x, dtype=np.float32)
    Wg_w = np.asarray(Wg_w, dtype=np.float32)
    Wg_b = np.asarray(Wg_b, dtype=np.float32)
    Wn_w = np.asarray(Wn_w, dtype=np.float32)
    Wn_b = np.asarray(Wn_b, dtype=np.float32)
    W1 = np.asarray(W1, dtype=np.float32)
    b1 = np.asarray(b1, dtype=np.float32)
    W2 = np.asarray(W2, dtype=np.float32)
    b2 = np.asarray(b2, dtype=np.float32)
    assert x.shape == (_B, _T, _D), x.shape

    x2 = np.ascontiguousarray(x.reshape(_N, _D))
    e1, e2, g1, g2 = _route(x2, Wg_w, Wg_b, Wn_w, Wn_b)

    # Entries: one (token, expert, gateweight) pair per routed slot.
    # Within each expert, slots are ordered by gate weight DESCENDING so
    # the last chunk holds the smallest-g slots — their mm1 runs in fp8
    # (error contribution is bounded by g * fp8-error), and any capacity
    # overflow (the very smallest g) falls back to the exact host path.
    ent_e = np.concatenate([e1, e2])
    ent_tok = np.concatenate([np.arange(_N), np.arange(_N)])
    ent_g = np.concatenate([g1, g2])
    perm = np.lexsort((-ent_g, ent_e))
    counts = np.bincount(ent_e, minlength=_E)
    starts = np.concatenate([[0], np.cumsum(counts)[:-1]])

    # Device capacity: fixed C slots per expert; entries past the cap
    # (expected ~1% of slots when counts exceed the mean) fall back to an
    # exact host-side FFN.
    C = _C
    nTT = C // 128

    # Global slot id for each entry (expert * C + position within expert);
    # overflow entries get the sentinel slot _E*C (a zero row on combine).
    pos_sorted = np.arange(2 * _N) - starts[ent_e[perm]]
    over = pos_sorted >= C
    slot_sorted = np.where(over, _E * C, ent_e[perm] * C + pos_sorted)
    slots = np.empty(2 * _N, dtype=np.int64)
    slots[perm] = slot_sorted
    tok_sorted = ent_tok[perm]

    # Per-slot gate weights, flattened over all cores.
    gflat = np.zeros(_E * C + 1, dtype=np.float32)
    gflat[slot_sorted] = ent_g[perm]
    gflat = gflat[:_E * C]

    cast = (lambda a: np.ascontiguousarray(a, dtype=ml_dtypes.bfloat16)) \
        if _MODE == "bf16" else (lambda a: np.ascontiguousarray(a, dtype=np.float32))

    fp8 = _FP8 and _MODE == "bf16"
    CH = 512
    in_maps = []
    for e in range(_E):
        cnt = min(int(counts[e]), C)
        toks = tok_sorted[starts[e]:starts[e] + cnt]
        xg = np.zeros((C, _D), dtype=np.float32)
        xg[:cnt] = x2[toks]
        im = {
            "xt": cast(xg[:C - CH].T if fp8 else xg.T),
            "w1": cast(W1[e]),
            # pre-arranged to the device SBUF layout [128, f, d]
            "w2t": cast(W2[e].reshape(_FF // 128, 128, _D)
                        .transpose(1, 0, 2).reshape(128, -1)),
            "bg": np.ascontiguousarray(np.concatenate([
                b1[e].reshape(_FF // 128, 128).T,
                gflat[e * C:(e + 1) * C].reshape(nTT, 128).T,
            ], axis=1), dtype=np.float32),
        }
        if fp8:
            im["xt8"] = np.ascontiguousarray(
                (xg[C - CH:].T * np.float32(8.0))
                .astype(ml_dtypes.float8_e4m3))
            im["w18"] = np.ascontiguousarray(
                (W1[e] * np.float32(64.0)).astype(ml_dtypes.float8_e4m3))
        in_maps.append(im)

    nc = _get_device_kernel(C, _MODE, fp8)
    res = run_bass_kernel_spmd(nc, in_maps, list(range(_E)), trace=_TRACE)
    LAST_RESULTS = res

    y_all = np.concatenate(
        [np.asarray(res.results[e]["y"], dtype=np.float32) for e in range(_E)]
        + [np.zeros((1, _D), dtype=np.float32)], axis=0)  # [E*C + 1, D]
    out = y_all[slots[:_N]] + y_all[slots[_N:]]

    # Exact host FFN for capacity-overflow entries (past slot C of an expert).
    if over.any():
        ov_tok = tok_sorted[over]
        ov_e = ent_e[perm][over]
        ov_g = ent_g[perm][over]
        for e in np.unique(ov_e):
            m = ov_e == e
            t = ov_tok[m]
            h = np.maximum(x2[t] @ W1[e] + b1[e], 0.0)
            out[t] += ov_g[m][:, None] * (h @ W2[e])

    # b2 of the chosen experts (device computes g*(relu(.)@W2) without b2)
    if b2.any():
        out += g1[:, None] * b2[e1] + g2[:, None] * b2[e2]
    return out.reshape(_B, _T, _D).astype(np.float32)

